# revision 48
# baseline (speedup 1.0000x reference)
"""Tensor-parallel GQA multi-head attention for 8 Trainium2 NeuronCores.

Sharding: (batch, kv-head) per core — core c handles batch b=c//4 and
kv group kvh=c%4 (4 query heads). Each core projects q/k/v for its
batch's 2048 tokens only, runs causal attention for its 4 heads, and
produces a full-D partial of the output projection; the host sums the
4 partials per batch (all-reduce after wo).

Per-core layout: activations transposed (feature dim on partitions,
tokens on the free axis); every matmul contracts over the partition dim:
  QT/KT/VT = W^T-chunks (lhsT) x xT (rhs)      [dh, tokens]
  S^T   = KT-chunk (lhsT) x QT (rhs)           [s, t]  (causal superblocks)
  P^T   = exp(S^T + causal mask)               (no max-subtraction: scores
                                                are bounded ~N(0, 1))
  l     = ones x P^T (column sums via PE)      [1, t]
  avT   = Vn-chunk (lhsT) x P^T (rhs)          [dh, t]; scaled by 1/l
  out   = avT-chunk (lhsT) x woT (rhs)         [t, d] partial, DMA'd bf16
"""

import numpy as np

B, T, D, H, KV = 2, 2048, 2048, 16, 4
DH = 128
NCORES = 8
HPC = H // KV              # 4 query heads per core (one kv group)
ND = D // 128              # 16 contraction chunks
NSB = T // 512             # 4 causal superblocks
ROPE_BASE = 10000.0
NEG = -30000.0
WSCALE = 128.0             # fp8 weight pre-scale (2^7)
WDESC = 1.0 / WSCALE
AVSCALE = 8.0              # fp8 av pre-scale (2^3)
ODESC = 1.0 / (WSCALE * AVSCALE)

_cache = {}


def _ensure_path():
    try:
        import concourse.bass  # noqa: F401
    except ImportError:
        import sys
        for p in ("/opt/trn_rl_repo", "/root/.axon_site/_ro/trn_rl_repo"):
            if p not in sys.path:
                sys.path.insert(0, p)
        import concourse.bass  # noqa: F401


def _split_multi_waits(nc, mybir, max_waits=1):
    """This container's walrus rejects >1 sync-wait on one instruction
    (seen on the Tile tail drain). Move extra waits onto preceding NoOps
    on the same engine; per-engine program order preserves semantics."""
    for bb in nc.main_func.blocks:
        new_insts = []
        for ins in bb.instructions:
            si = getattr(ins, "sync_info", None)
            if si is not None and si.on_wait and len(si.on_wait) > max_waits:
                waits = list(si.on_wait)
                extra, keep = waits[:-max_waits], waits[-max_waits:]
                for w in extra:
                    new_insts.append(
                        mybir.InstNoOp(
                            name=nc.get_next_instruction_name(),
                            sync_info=mybir.SyncInfo(on_wait=[w], on_update=[]),
                            bass_nofuse=True,
                            engine=ins.engine,
                            ins=[],
                            outs=[],
                        )
                    )
                si.on_wait = keep
            new_insts.append(ins)
        bb.instructions = new_insts


def _build(split_waits=True):
    _ensure_path()
    import concourse.bass as bass
    import concourse.mybir as mybir
    import concourse.tile as tile
    from concourse.masks import make_identity

    f32 = mybir.dt.float32
    bf16 = mybir.dt.bfloat16
    fr = mybir.dt.float32r
    nc = bass.Bass()

    f8 = mybir.dt.float8e4
    DR = mybir.MatmulPerfMode.DoubleRow
    # fp8 hi/lo residual pairs: w ~ w_hi + w_lo with both operands of every
    # GEMM term in fp8 so the PE runs DoubleRow (2 K-chunks per instr at 0.5
    # cycles/row). 3 terms (hi*hi + hi*lo + lo*hi) = 0.75x the f32r cost with
    # ~0.3% error. Weights are pre-scaled by 2^7 on the host so w_lo lands
    # mostly in fp8 normals; the PSUM drain rescales by 2^-7 for free.
    xT = nc.declare_dram_parameter("xT", [D, T], bf16, isOutput=False)
    xH = nc.declare_dram_parameter("xH", [D, T], f8, isOutput=False)
    xL = nc.declare_dram_parameter("xL", [D, T], f8, isOutput=False)
    wqH = nc.declare_dram_parameter("wqH", [D, HPC * DH], f8, isOutput=False)
    wqL = nc.declare_dram_parameter("wqL", [D, HPC * DH], f8, isOutput=False)
    wkH = nc.declare_dram_parameter("wkH", [D, DH], f8, isOutput=False)
    wkL = nc.declare_dram_parameter("wkL", [D, DH], f8, isOutput=False)
    wvT = nc.declare_dram_parameter("wvT", [D, DH], bf16, isOutput=False)
    woH = nc.declare_dram_parameter("woH", [HPC * DH, D], f8, isOutput=False)
    woL = nc.declare_dram_parameter("woL", [HPC * DH, D], f8, isOutput=False)
    cosT = nc.declare_dram_parameter("cosT", [DH, T], f32, isOutput=False)
    sinT = nc.declare_dram_parameter("sinT", [DH, T], f32, isOutput=False)
    out = nc.declare_dram_parameter("out", [T, D], bf16, isOutput=True)

    with nc.allow_low_precision(reason="float32r fast matmul path"), \
         tile.TileContext(nc) as tc:
        with tc.tile_pool(name="persist", bufs=1) as P:
            ident = P.tile([128, 128], f32, tag="ident")
            maskT = P.tile([128, 128], f32, tag="maskT")
            ones = P.tile([128, 1], fr, tag="ones")
            ones_r = P.tile([1, 128], fr, tag="ones_r")
            ones_f = P.tile([128, 1], f32, tag="ones_f")
            ones_rf = P.tile([1, 128], f32, tag="ones_rf")
            ident_r = P.tile([128, 128], fr, tag="ident_r")
            make_identity(nc, ident[:])
            nc.vector.tensor_copy(ident_r[:], ident[:])
            # S^T diag block mask: keep (s_local - t_local) <= 0, else NEG
            nc.gpsimd.memset(maskT[:], 0.0)
            nc.gpsimd.affine_select(
                out=maskT[:],
                in_=maskT[:],
                compare_op=mybir.AluOpType.is_ge,
                fill=NEG,
                base=0,
                pattern=[[1, 128]],
                channel_multiplier=-1,
            )
            nc.gpsimd.memset(ones_f[:], 1.0)
            # rbc broadcast carries the fp8 av pre-scale: AVT holds av*AVSCALE
            nc.gpsimd.memset(ones_rf[:], AVSCALE)
            nc.vector.tensor_copy(ones[:], ones_f[:])
            nc.vector.tensor_copy(ones_r[:], ones_rf[:])

            QT = [P.tile([128, T], fr, tag=f"qt{h}", name=f"qt{h}") for h in range(HPC)]
            KT = P.tile([128, T], fr, tag="kt")
            Vn = P.tile([128, T], fr, tag="vn")
            AVT = [P.tile([128, T], fr, tag=f"avt{h}", name=f"avt{h}") for h in range(HPC)]
            av_hi = P.tile([128, HPC * T], f8, tag="avhi")
            av_lo = P.tile([128, HPC * T], f8, tag="avlo")

            # ---------- phase A: QKV projections + RoPE ------
            # x/wq/wk/wv in bf16 (same PE rate as f32r, half the DMA). V is
            # projected directly into natural [s, dh] layout (lhsT = x chunk,
            # rhs = wv chunk) so no PE transposes are needed. RoPE for tile
            # tq5 is emitted during tile tq5+1's chain matmuls so the PSUM
            # drain never stalls the PE.
            with tc.tile_pool(name="wpool", bufs=1) as WP, \
                 tc.tile_pool(name="xp", bufs=5) as XP, \
                 tc.tile_pool(name="x8p", bufs=6) as X8, \
                 tc.tile_pool(name="ropetA", bufs=4) as RT2, \
                 tc.tile_pool(name="psA", bufs=1, space="PSUM") as PSA, \
                 tc.tile_pool(name="psScrA", bufs=2, space="PSUM") as PSCR:
                wq_hi = WP.tile([128, ND * HPC * DH], f8, tag="wqh")
                wq_lo = WP.tile([128, ND * HPC * DH], f8, tag="wql")
                wk_hi = WP.tile([128, ND * DH], f8, tag="wkh")
                wk_lo = WP.tile([128, ND * DH], f8, tag="wkl")
                wv_sb = WP.tile([128, ND * DH], bf16, tag="wv")
                cos_sb = WP.tile([128, T], f32, tag="cos")
                sin_sb = WP.tile([128, T], f32, tag="sin")

                def load_weight_quarter(qi):
                    lo, hi = qi * (ND // 4), (qi + 1) * (ND // 4)
                    for sb, dr in ((wq_hi, wqH), (wq_lo, wqL)):
                        nc.sync.dma_start(
                            out=sb[:, lo * 512: hi * 512].rearrange(
                                "p (c m) -> p c m", c=hi - lo),
                            in_=dr[lo * 128: hi * 128, :].rearrange(
                                "(c p) m -> p c m", p=128))
                    for sb, dr in ((wk_hi, wkH), (wk_lo, wkL), (wv_sb, wvT)):
                        nc.sync.dma_start(
                            out=sb[:, lo * 128: hi * 128].rearrange(
                                "p (c m) -> p c m", c=hi - lo),
                            in_=dr[lo * 128: hi * 128, :].rearrange(
                                "(c p) m -> p c m", p=128))

                # keep-warm matmuls: ramp the PE while weight/x DMAs land
                for _ in range(40):
                    wps = PSCR.tile([128, 512], f32, tag="scr", name="warm")
                    nc.tensor.matmul(wps[:, 0:128], lhsT=ident[:], rhs=ident[:],
                                     start=True, stop=True)

                load_weight_quarter(0)

                def emit_rope(tcol, pq, pk):
                    """Drain all 5 psum banks first (split Act/DVE, ~2us) so
                    the next tile's chains never wait, then apply rope from
                    SBUF: rotate-half via partition-offset reads (no PE
                    matmul), one half on Pool to keep DVE under the PE rate."""
                    mul = mybir.AluOpType.mult
                    nc.scalar.mul(QT[0][:, tcol], pq[0][:, :], WDESC)
                    nc.vector.tensor_scalar_mul(QT[1][:, tcol], pq[1][:, :], WDESC)
                    nc.scalar.mul(QT[2][:, tcol], pq[2][:, :], WDESC)
                    nc.vector.tensor_scalar_mul(QT[3][:, tcol], pq[3][:, :], WDESC)
                    nc.scalar.mul(KT[:, tcol], pk[:, :], WDESC)
                    for tgt in QT + [KT]:
                        # sin rows are duplicated halves, so read sin at the
                        # same base partition as the shifted q input (walrus
                        # requires equal input base partitions in SBUF)
                        rtmp = RT2.tile([128, 512], f32, tag="rtmp")
                        nc.vector.scalar_tensor_tensor(
                            out=rtmp[0:64, :], in0=tgt[64:128, tcol], scalar=-1.0,
                            in1=sin_sb[64:128, tcol], op0=mul, op1=mul)
                        nc.gpsimd.tensor_mul(rtmp[64:128, :], tgt[0:64, tcol],
                                             sin_sb[0:64, tcol])
                        nc.vector.tensor_mul(tgt[:, tcol], tgt[:, tcol],
                                             cos_sb[:, tcol])
                        nc.gpsimd.tensor_add(tgt[:, tcol], tgt[:, tcol], rtmp[:])
                for tq5 in range(T // 512):
                    pq = [PSA.tile([128, 512], f32, tag=f"pq{h}", name=f"pq{h}")
                          for h in range(HPC)]
                    pk = PSA.tile([128, 512], f32, tag="pk")
                    pv = PSA.tile([128, 512], f32, tag="pv")
                    xts = []
                    for dcg in range(4):
                        tslc = slice(tq5 * 512, (tq5 + 1) * 512)
                        dslc = slice(dcg * 512, (dcg + 1) * 512)
                        xt = XP.tile([128, 4 * 512], bf16, tag="x")
                        xts.append(xt)
                        nc.sync.dma_start(
                            out=xt[:].rearrange("p (c m) -> p c m", c=4),
                            in_=xT[dslc, tslc].rearrange("(c p) m -> p c m",
                                                         p=128))
                        xh = X8.tile([128, 4 * 512], f8, tag="xh")
                        nc.sync.dma_start(
                            out=xh[:].rearrange("p (c m) -> p c m", c=4),
                            in_=xH[dslc, tslc].rearrange("(c p) m -> p c m",
                                                         p=128))
                        xl = X8.tile([128, 4 * 512], f8, tag="xl")
                        nc.sync.dma_start(
                            out=xl[:].rearrange("p (c m) -> p c m", c=4),
                            in_=xL[dslc, tslc].rearrange("(c p) m -> p c m",
                                                         p=128))
                        if tq5 == 0 and dcg < 3:
                            load_weight_quarter(dcg + 1)
                        if tq5 == 0 and dcg == 3:
                            nc.sync.dma_start(out=cos_sb[:], in_=cosT[:, :])
                            nc.sync.dma_start(out=sin_sb[:], in_=sinT[:, :])
                        # q/k chains: 3-term hi/lo fp8 DoubleRow over chunk
                        # pairs (2 pairs per dcg)
                        xh2 = xh[:].rearrange("p (c m) -> p c m", c=4)
                        xl2 = xl[:].rearrange("p (c m) -> p c m", c=4)
                        for dcp in range(2):
                            dc = dcg * 4 + dcp * 2
                            st, sp0 = (dc == 0), (dc == ND - 2)
                            xpair_hh = xh2[:, dcp * 2:dcp * 2 + 2, :]
                            xpair_ll = xl2[:, dcp * 2:dcp * 2 + 2, :]
                            wqh4 = wq_hi[:].rearrange("p (c m) -> p c m", c=ND)
                            wql4 = wq_lo[:].rearrange("p (c m) -> p c m", c=ND)
                            for h in range(HPC):
                                hs = slice(h * 128, (h + 1) * 128)
                                whp = wqh4[:, dc:dc + 2, hs]
                                wlp = wql4[:, dc:dc + 2, hs]
                                nc.tensor.matmul(pq[h][:], lhsT=whp, rhs=xpair_hh,
                                                 start=st, stop=False, perf_mode=DR)
                                nc.tensor.matmul(pq[h][:], lhsT=whp, rhs=xpair_ll,
                                                 start=False, stop=False, perf_mode=DR)
                                nc.tensor.matmul(pq[h][:], lhsT=wlp, rhs=xpair_hh,
                                                 start=False, stop=sp0, perf_mode=DR)
                            khp = wk_hi[:, dc * 128:(dc + 2) * 128].rearrange(
                                "p (c m) -> p c m", c=2)
                            klp = wk_lo[:, dc * 128:(dc + 2) * 128].rearrange(
                                "p (c m) -> p c m", c=2)
                            nc.tensor.matmul(pk[:], lhsT=khp, rhs=xpair_hh,
                                             start=st, stop=False, perf_mode=DR)
                            nc.tensor.matmul(pk[:], lhsT=khp, rhs=xpair_ll,
                                             start=False, stop=False, perf_mode=DR)
                            nc.tensor.matmul(pk[:], lhsT=klp, rhs=xpair_hh,
                                             start=False, stop=sp0, perf_mode=DR)
                    # V directly in natural [s, dh] layout: lhsT = x chunk
                    # (stationary), rhs = wv chunk (moving, 128 wide)
                    for scl in range(4):
                        for dcg in range(4):
                            for dci in range(4):
                                dc = dcg * 4 + dci
                                nc.tensor.matmul(
                                    pv[:, scl * 128:(scl + 1) * 128],
                                    lhsT=xts[dcg][:, dci * 512 + scl * 128:
                                                  dci * 512 + (scl + 1) * 128],
                                    rhs=wv_sb[:, dc * 128:(dc + 1) * 128],
                                    start=(dc == 0), stop=(dc == ND - 1))
                    tcol = slice(tq5 * 512, (tq5 + 1) * 512)
                    # fused drain+rope on DVE/Pool (PE rolls straight into the
                    # next tile's chains); Vn drain on the idle Act engine
                    nc.scalar.copy(Vn[:, tcol], pv[:, :])
                    emit_rope(tcol, pq, pk)
            # ---------- phase B: attention + interleaved output projection --
            with tc.tile_pool(name="wop", bufs=1) as WOP, \
                 tc.tile_pool(name="ptp", bufs=4) as PTP, \
                 tc.tile_pool(name="rrp", bufs=2) as RRP, \
                 tc.tile_pool(name="osbp", bufs=3) as OSBP, \
                 tc.tile_pool(name="psSt", bufs=4, space="PSUM") as PSST, \
                 tc.tile_pool(name="psL", bufs=1, space="PSUM") as PSL, \
                 tc.tile_pool(name="psAv", bufs=1, space="PSUM") as PSAV, \
                 tc.tile_pool(name="psC", bufs=2, space="PSUM") as PSC:
                wo_hi = WOP.tile([128, HPC * D], f8, tag="woh")
                wo_lo = WOP.tile([128, HPC * D], f8, tag="wol")
                for sb, dr in ((wo_hi, woH), (wo_lo, woL)):
                    nc.sync.dma_start(
                        out=sb[:].rearrange("p (c n) -> p c n", c=HPC),
                        in_=dr[:, :].rearrange("(c p) n -> p c n", p=128))

                def emit_wo(tsb, tj):
                    tcx = (tsb * 512) // 128 + tj
                    avh = av_hi[:].rearrange("p (c m) -> p c m", c=HPC)
                    avl = av_lo[:].rearrange("p (c m) -> p c m", c=HPC)
                    wh = wo_hi[:].rearrange("p (c n) -> p c n", c=HPC)
                    wl = wo_lo[:].rearrange("p (c n) -> p c n", c=HPC)
                    tsl = slice(tcx * 128, (tcx + 1) * 128)
                    for dq in range(4):
                        dsl = slice(dq * 512, (dq + 1) * 512)
                        wo_ps = PSC.tile([128, D // 4], f32, tag="wops")
                        for hp in (0, 2):
                            hsl = slice(hp, hp + 2)
                            nc.tensor.matmul(
                                wo_ps[:], lhsT=avh[:, hsl, tsl],
                                rhs=wh[:, hsl, dsl],
                                start=(hp == 0), stop=False, perf_mode=DR)
                            nc.tensor.matmul(
                                wo_ps[:], lhsT=avh[:, hsl, tsl],
                                rhs=wl[:, hsl, dsl],
                                start=False, stop=False, perf_mode=DR)
                            nc.tensor.matmul(
                                wo_ps[:], lhsT=avl[:, hsl, tsl],
                                rhs=wh[:, hsl, dsl],
                                start=False, stop=(hp == 2), perf_mode=DR)
                        osb = OSBP.tile([128, D // 4], bf16, tag="osb")
                        if (tj + dq) % 2 == 0:
                            nc.vector.tensor_scalar_mul(osb[:], wo_ps[:], ODESC)
                        else:
                            nc.scalar.mul(osb[:], wo_ps[:], ODESC)
                        nc.sync.dma_start(
                            out=out[tsl, dsl], in_=osb[:])

                # deepest group first (best pipelining while nothing overlaps);
                # each group's output projection is emitted one group late so
                # it fills the next group's exp-latency stalls
                prev_tsb = None
                for tsb in [2, 3, 1, 0]:
                    n_sc = (tsb + 1) * 4
                    tg = slice(tsb * 512, (tsb + 1) * 512)
                    for h in range(HPC):
                        if prev_tsb is not None:
                            emit_wo(prev_tsb, h)
                        av_ps = PSAV.tile([128, 512], f32, tag="av")
                        l_ps = PSL.tile([128, 512], f32, tag="l")
                        for sc in range(n_sc):
                            sc_rel = sc - tsb * 4
                            c0 = max(sc_rel, 0) * 128   # first valid t col
                            # keep the moving operand >= 256 wide: float32r
                            # matmuls under 256 run at 1/4 rate
                            c0w = min(c0, 256)
                            nv = slice(c0w, 512)
                            tgn = slice(tsb * 512 + c0w, (tsb + 1) * 512)
                            st_ps = PSST.tile([128, 512], f32, tag="st")
                            nc.tensor.matmul(
                                st_ps[:, nv],
                                lhsT=KT[:, sc * 128:(sc + 1) * 128],
                                rhs=QT[h][:, tgn], start=True, stop=True)
                            if sc_rel >= 0:
                                blk = st_ps[:, c0:c0 + 128]
                                nc.vector.tensor_add(blk, blk, maskT[:])
                            if c0 > c0w:
                                # widened prefix is fully below the diagonal
                                nc.vector.tensor_scalar_add(
                                    st_ps[:, c0w:c0], st_ps[:, c0w:c0], NEG)
                            pt = PTP.tile([128, 512], fr, tag="pt")
                            nc.scalar.activation(
                                pt[:, nv], st_ps[:, nv],
                                mybir.ActivationFunctionType.Exp)
                            nc.tensor.matmul(
                                l_ps[0:1, nv], lhsT=ones[:], rhs=pt[:, nv],
                                start=(sc == 0), stop=(sc == n_sc - 1))
                            nc.tensor.matmul(
                                av_ps[:, nv],
                                lhsT=Vn[:, sc * 128:(sc + 1) * 128],
                                rhs=pt[:, nv], start=(sc == 0),
                                stop=(sc == n_sc - 1))
                        rr = RRP.tile([1, 512], fr, tag="rr")
                        nc.vector.reciprocal(rr[:], l_ps[0:1, :])
                        nc.vector.tensor_copy(AVT[h][:, tg], av_ps[:])
                        # rbc shares the l_ps bank (their lifetimes are
                        # naturally sequential), keeping wops free for wo
                        rbc = PSL.tile([128, 512], f32, tag="l", name="rbc")
                        nc.tensor.matmul(
                            rbc[:], lhsT=ones_r[:], rhs=rr[:],
                            start=True, stop=True)
                        nc.vector.tensor_mul(AVT[h][:, tg], AVT[h][:, tg], rbc[:])
                        # hi/lo fp8 decompose of av*AVSCALE for the fp8 wo
                        hsl8 = slice(h * T + tsb * 512, h * T + (tsb + 1) * 512)
                        nc.scalar.copy(av_hi[:, hsl8], AVT[h][:, tg])
                        nc.vector.tensor_sub(av_lo[:, hsl8], AVT[h][:, tg],
                                             av_hi[:, hsl8])
                    prev_tsb = tsb
                # last group's output projection runs at the tail
                for tj in range(4):
                    emit_wo(prev_tsb, tj)

    if split_waits:
        _split_multi_waits(nc, mybir)
    return nc


def _host_inputs(x, wq, wk, wv, wo):
    half = DH // 2
    inv = (1.0 / (ROPE_BASE ** (np.arange(half, dtype=np.float32) / half))).astype(np.float32)
    ang = np.arange(T, dtype=np.float32)[:, None] * inv[None, :]          # (T, 64)
    c = np.cos(ang).T.astype(np.float32)                                  # (64, T)
    s = np.sin(ang).T.astype(np.float32)
    cosT = np.ascontiguousarray(np.concatenate([c, c], axis=0))           # (128, T)
    sinT = np.ascontiguousarray(np.concatenate([s, s], axis=0))
    scale = np.float32(1.0 / np.sqrt(DH))
    import ml_dtypes
    bf = ml_dtypes.bfloat16
    f8 = ml_dtypes.float8_e4m3

    def hilo(a):
        hi = a.astype(f8)
        lo = (a - hi.astype(np.float32)).astype(f8)
        return np.ascontiguousarray(hi), np.ascontiguousarray(lo)

    in_maps = []
    for core in range(NCORES):
        b, kvh = core // KV, core % KV
        hlo = kvh * HPC * DH
        xb = x[b].T
        xh, xl = hilo(xb)
        wqh, wql = hilo((wq[hlo:hlo + HPC * DH, :] * (scale * WSCALE)).T)
        wkh, wkl = hilo(wk[kvh * DH:(kvh + 1) * DH, :].T * WSCALE)
        woh, wol = hilo(wo[:, hlo:hlo + HPC * DH].T * WSCALE)
        in_maps.append({
            "xT": np.ascontiguousarray(xb.astype(bf)),
            "xH": xh, "xL": xl,
            "wqH": wqh, "wqL": wql,
            "wkH": wkh, "wkL": wkl,
            "wvT": np.ascontiguousarray(wv[kvh * DH:(kvh + 1) * DH, :].T.astype(bf)),
            "woH": woh, "woL": wol,
            "cosT": cosT,
            "sinT": sinT,
        })
    return in_maps


def kernel(x, wq, wk, wv, wo):
    _ensure_path()
    from concourse.bass_utils import run_bass_kernel_spmd

    x = np.asarray(x, dtype=np.float32)
    wq = np.asarray(wq, dtype=np.float32)
    wk = np.asarray(wk, dtype=np.float32)
    wv = np.asarray(wv, dtype=np.float32)
    wo = np.asarray(wo, dtype=np.float32)

    if "nc" not in _cache:
        _cache["nc"] = _build()
    nc = _cache["nc"]

    in_maps = _host_inputs(x, wq, wk, wv, wo)
    res = run_bass_kernel_spmd(nc, in_maps, list(range(NCORES)))
    out = np.zeros((B, T, D), dtype=np.float32)
    for core in range(NCORES):
        b = core // KV
        out[b] += res.results[core]["out"].astype(np.float32)
    return out


# revision 49
# speedup vs baseline: 1.0340x; 1.0340x over previous
"""Tensor-parallel GQA multi-head attention for 8 Trainium2 NeuronCores.

Sharding: (batch, kv-head) per core — core c handles batch b=c//4 and
kv group kvh=c%4 (4 query heads). Each core projects q/k/v for its
batch's 2048 tokens only, runs causal attention for its 4 heads, and
produces a full-D partial of the output projection; the host sums the
4 partials per batch (all-reduce after wo).

Per-core layout: activations transposed (feature dim on partitions,
tokens on the free axis); every matmul contracts over the partition dim:
  QT/KT/VT = W^T-chunks (lhsT) x xT (rhs)      [dh, tokens]
  S^T   = KT-chunk (lhsT) x QT (rhs)           [s, t]  (causal superblocks)
  P^T   = exp(S^T + causal mask)               (no max-subtraction: scores
                                                are bounded ~N(0, 1))
  l     = ones x P^T (column sums via PE)      [1, t]
  avT   = Vn-chunk (lhsT) x P^T (rhs)          [dh, t]; scaled by 1/l
  out   = avT-chunk (lhsT) x woT (rhs)         [t, d] partial, DMA'd bf16
"""

import numpy as np

B, T, D, H, KV = 2, 2048, 2048, 16, 4
DH = 128
NCORES = 8
HPC = H // KV              # 4 query heads per core (one kv group)
ND = D // 128              # 16 contraction chunks
NSB = T // 512             # 4 causal superblocks
ROPE_BASE = 10000.0
NEG = -30000.0
WSCALE = 128.0             # fp8 weight pre-scale (2^7)
WDESC = 1.0 / WSCALE
AVSCALE = 8.0              # fp8 av pre-scale (2^3)
ODESC = 1.0 / (WSCALE * AVSCALE)

_cache = {}


def _ensure_path():
    try:
        import concourse.bass  # noqa: F401
    except ImportError:
        import sys
        for p in ("/opt/trn_rl_repo", "/root/.axon_site/_ro/trn_rl_repo"):
            if p not in sys.path:
                sys.path.insert(0, p)
        import concourse.bass  # noqa: F401


def _split_multi_waits(nc, mybir, max_waits=1):
    """This container's walrus rejects >1 sync-wait on one instruction
    (seen on the Tile tail drain). Move extra waits onto preceding NoOps
    on the same engine; per-engine program order preserves semantics."""
    for bb in nc.main_func.blocks:
        new_insts = []
        for ins in bb.instructions:
            si = getattr(ins, "sync_info", None)
            if si is not None and si.on_wait and len(si.on_wait) > max_waits:
                waits = list(si.on_wait)
                extra, keep = waits[:-max_waits], waits[-max_waits:]
                for w in extra:
                    new_insts.append(
                        mybir.InstNoOp(
                            name=nc.get_next_instruction_name(),
                            sync_info=mybir.SyncInfo(on_wait=[w], on_update=[]),
                            bass_nofuse=True,
                            engine=ins.engine,
                            ins=[],
                            outs=[],
                        )
                    )
                si.on_wait = keep
            new_insts.append(ins)
        bb.instructions = new_insts


def _build(split_waits=True):
    _ensure_path()
    import concourse.bass as bass
    import concourse.mybir as mybir
    import concourse.tile as tile
    from concourse.masks import make_identity

    f32 = mybir.dt.float32
    bf16 = mybir.dt.bfloat16
    fr = mybir.dt.float32r
    nc = bass.Bass()

    f8 = mybir.dt.float8e4
    DR = mybir.MatmulPerfMode.DoubleRow
    # fp8 hi/lo residual pairs: w ~ w_hi + w_lo with both operands of every
    # GEMM term in fp8 so the PE runs DoubleRow (2 K-chunks per instr at 0.5
    # cycles/row). 3 terms (hi*hi + hi*lo + lo*hi) = 0.75x the f32r cost with
    # ~0.3% error. Weights are pre-scaled by 2^7 on the host so w_lo lands
    # mostly in fp8 normals; the PSUM drain rescales by 2^-7 for free.
    xH = nc.declare_dram_parameter("xH", [D, T], f8, isOutput=False)
    xL = nc.declare_dram_parameter("xL", [D, T], f8, isOutput=False)
    wqH = nc.declare_dram_parameter("wqH", [D, HPC * DH], f8, isOutput=False)
    wqL = nc.declare_dram_parameter("wqL", [D, HPC * DH], f8, isOutput=False)
    wkH = nc.declare_dram_parameter("wkH", [D, DH], f8, isOutput=False)
    wkL = nc.declare_dram_parameter("wkL", [D, DH], f8, isOutput=False)
    wvH = nc.declare_dram_parameter("wvH", [D, DH], f8, isOutput=False)
    wvL = nc.declare_dram_parameter("wvL", [D, DH], f8, isOutput=False)
    woH = nc.declare_dram_parameter("woH", [HPC * DH, D], f8, isOutput=False)
    woL = nc.declare_dram_parameter("woL", [HPC * DH, D], f8, isOutput=False)
    cosT = nc.declare_dram_parameter("cosT", [DH, T], f32, isOutput=False)
    sinT = nc.declare_dram_parameter("sinT", [DH, T], f32, isOutput=False)
    out = nc.declare_dram_parameter("out", [T, D], bf16, isOutput=True)

    with nc.allow_low_precision(reason="float32r fast matmul path"), \
         tile.TileContext(nc) as tc:
        with tc.tile_pool(name="persist", bufs=1) as P:
            ident = P.tile([128, 128], f32, tag="ident")
            maskT = P.tile([128, 128], f32, tag="maskT")
            ones = P.tile([128, 1], fr, tag="ones")
            ones_r = P.tile([1, 128], fr, tag="ones_r")
            ones_f = P.tile([128, 1], f32, tag="ones_f")
            ones_rf = P.tile([1, 128], f32, tag="ones_rf")
            ident_r = P.tile([128, 128], fr, tag="ident_r")
            make_identity(nc, ident[:])
            nc.vector.tensor_copy(ident_r[:], ident[:])
            # S^T diag block mask: keep (s_local - t_local) <= 0, else NEG
            nc.gpsimd.memset(maskT[:], 0.0)
            nc.gpsimd.affine_select(
                out=maskT[:],
                in_=maskT[:],
                compare_op=mybir.AluOpType.is_ge,
                fill=NEG,
                base=0,
                pattern=[[1, 128]],
                channel_multiplier=-1,
            )
            nc.gpsimd.memset(ones_f[:], 1.0)
            # rbc broadcast carries the fp8 av pre-scale: AVT holds av*AVSCALE
            nc.gpsimd.memset(ones_rf[:], AVSCALE)
            nc.vector.tensor_copy(ones[:], ones_f[:])
            nc.vector.tensor_copy(ones_r[:], ones_rf[:])

            QT = [P.tile([128, T], fr, tag=f"qt{h}", name=f"qt{h}") for h in range(HPC)]
            KT = P.tile([128, T], fr, tag="kt")
            Vn = P.tile([128, T], fr, tag="vn")
            AVT = [P.tile([128, T], fr, tag=f"avt{h}", name=f"avt{h}") for h in range(HPC)]
            av_hi = P.tile([128, HPC * T], f8, tag="avhi")
            av_lo = P.tile([128, HPC * T], f8, tag="avlo")

            # ---------- phase A: QKV projections + RoPE ------
            # x/wq/wk/wv in bf16 (same PE rate as f32r, half the DMA). V is
            # projected directly into natural [s, dh] layout (lhsT = x chunk,
            # rhs = wv chunk) so no PE transposes are needed. RoPE for tile
            # tq5 is emitted during tile tq5+1's chain matmuls so the PSUM
            # drain never stalls the PE.
            with tc.tile_pool(name="wpool", bufs=1) as WP, \
                 tc.tile_pool(name="xp", bufs=5) as XP, \
                 tc.tile_pool(name="x8p", bufs=12) as X8, \
                 tc.tile_pool(name="ropetA", bufs=4) as RT2, \
                 tc.tile_pool(name="psA", bufs=1, space="PSUM") as PSA, \
                 tc.tile_pool(name="psScrA", bufs=2, space="PSUM") as PSCR:
                wq_hi = WP.tile([128, ND * HPC * DH], f8, tag="wqh")
                wq_lo = WP.tile([128, ND * HPC * DH], f8, tag="wql")
                wk_hi = WP.tile([128, ND * DH], f8, tag="wkh")
                wk_lo = WP.tile([128, ND * DH], f8, tag="wkl")
                wv_hi = WP.tile([128, ND * DH], f8, tag="wvh")
                wv_lo = WP.tile([128, ND * DH], f8, tag="wvl")
                cos_sb = WP.tile([128, T], f32, tag="cos")
                sin_sb = WP.tile([128, T], f32, tag="sin")

                def load_weight_quarter(qi):
                    lo, hi = qi * (ND // 4), (qi + 1) * (ND // 4)
                    for sb, dr in ((wq_hi, wqH), (wq_lo, wqL)):
                        nc.sync.dma_start(
                            out=sb[:, lo * 512: hi * 512].rearrange(
                                "p (c m) -> p c m", c=hi - lo),
                            in_=dr[lo * 128: hi * 128, :].rearrange(
                                "(c p) m -> p c m", p=128))
                    for sb, dr in ((wk_hi, wkH), (wk_lo, wkL), (wv_hi, wvH), (wv_lo, wvL)):
                        nc.sync.dma_start(
                            out=sb[:, lo * 128: hi * 128].rearrange(
                                "p (c m) -> p c m", c=hi - lo),
                            in_=dr[lo * 128: hi * 128, :].rearrange(
                                "(c p) m -> p c m", p=128))

                # keep-warm matmuls: ramp the PE while weight/x DMAs land
                for _ in range(40):
                    wps = PSCR.tile([128, 512], f32, tag="scr", name="warm")
                    nc.tensor.matmul(wps[:, 0:128], lhsT=ident[:], rhs=ident[:],
                                     start=True, stop=True)

                load_weight_quarter(0)

                def emit_rope(tcol, pq, pk, pv):
                    """Drain all 5 psum banks first (split Act/DVE, ~2us) so
                    the next tile's chains never wait, then apply rope from
                    SBUF: rotate-half via partition-offset reads (no PE
                    matmul), one half on Pool to keep DVE under the PE rate."""
                    mul = mybir.AluOpType.mult
                    nc.scalar.mul(QT[0][:, tcol], pq[0][:, :], WDESC)
                    nc.vector.tensor_scalar_mul(QT[1][:, tcol], pq[1][:, :], WDESC)
                    nc.scalar.mul(QT[2][:, tcol], pq[2][:, :], WDESC)
                    nc.vector.tensor_scalar_mul(QT[3][:, tcol], pq[3][:, :], WDESC)
                    nc.scalar.mul(KT[:, tcol], pk[:, :], WDESC)
                    # Vn drain last on Act: the chain banks must free first
                    nc.scalar.mul(Vn[:, tcol], pv[:, :], WDESC)
                    for tgt in QT + [KT]:
                        # sin rows are duplicated halves, so read sin at the
                        # same base partition as the shifted q input (walrus
                        # requires equal input base partitions in SBUF)
                        rtmp = RT2.tile([128, 512], f32, tag="rtmp")
                        nc.vector.scalar_tensor_tensor(
                            out=rtmp[0:64, :], in0=tgt[64:128, tcol], scalar=-1.0,
                            in1=sin_sb[64:128, tcol], op0=mul, op1=mul)
                        nc.gpsimd.tensor_mul(rtmp[64:128, :], tgt[0:64, tcol],
                                             sin_sb[0:64, tcol])
                        nc.vector.tensor_mul(tgt[:, tcol], tgt[:, tcol],
                                             cos_sb[:, tcol])
                        nc.gpsimd.tensor_add(tgt[:, tcol], tgt[:, tcol], rtmp[:])
                for tq5 in range(T // 512):
                    pq = [PSA.tile([128, 512], f32, tag=f"pq{h}", name=f"pq{h}")
                          for h in range(HPC)]
                    pk = PSA.tile([128, 512], f32, tag="pk")
                    pv = PSA.tile([128, 512], f32, tag="pv")
                    xts = []
                    for dcg in range(4):
                        tslc = slice(tq5 * 512, (tq5 + 1) * 512)
                        dslc = slice(dcg * 512, (dcg + 1) * 512)
                        xh = X8.tile([128, 4 * 512], f8, tag="xh")
                        nc.sync.dma_start(
                            out=xh[:].rearrange("p (c m) -> p c m", c=4),
                            in_=xH[dslc, tslc].rearrange("(c p) m -> p c m",
                                                         p=128))
                        xl = X8.tile([128, 4 * 512], f8, tag="xl")
                        nc.sync.dma_start(
                            out=xl[:].rearrange("p (c m) -> p c m", c=4),
                            in_=xL[dslc, tslc].rearrange("(c p) m -> p c m",
                                                         p=128))
                        xts.append((xh, xl))
                        if tq5 == 0 and dcg < 3:
                            load_weight_quarter(dcg + 1)
                        if tq5 == 0 and dcg == 3:
                            nc.sync.dma_start(out=cos_sb[:], in_=cosT[:, :])
                            nc.sync.dma_start(out=sin_sb[:], in_=sinT[:, :])
                        # q/k chains: 3-term hi/lo fp8 DoubleRow over chunk
                        # pairs (2 pairs per dcg)
                        xh2 = xh[:].rearrange("p (c m) -> p c m", c=4)
                        xl2 = xl[:].rearrange("p (c m) -> p c m", c=4)
                        for dcp in range(2):
                            dc = dcg * 4 + dcp * 2
                            st, sp0 = (dc == 0), (dc == ND - 2)
                            xpair_hh = xh2[:, dcp * 2:dcp * 2 + 2, :]
                            xpair_ll = xl2[:, dcp * 2:dcp * 2 + 2, :]
                            wqh4 = wq_hi[:].rearrange("p (c m) -> p c m", c=ND)
                            wql4 = wq_lo[:].rearrange("p (c m) -> p c m", c=ND)
                            for h in range(HPC):
                                hs = slice(h * 128, (h + 1) * 128)
                                whp = wqh4[:, dc:dc + 2, hs]
                                wlp = wql4[:, dc:dc + 2, hs]
                                nc.tensor.matmul(pq[h][:], lhsT=whp, rhs=xpair_hh,
                                                 start=st, stop=False, perf_mode=DR)
                                nc.tensor.matmul(pq[h][:], lhsT=whp, rhs=xpair_ll,
                                                 start=False, stop=False, perf_mode=DR)
                                nc.tensor.matmul(pq[h][:], lhsT=wlp, rhs=xpair_hh,
                                                 start=False, stop=sp0, perf_mode=DR)
                            khp = wk_hi[:, dc * 128:(dc + 2) * 128].rearrange(
                                "p (c m) -> p c m", c=2)
                            klp = wk_lo[:, dc * 128:(dc + 2) * 128].rearrange(
                                "p (c m) -> p c m", c=2)
                            nc.tensor.matmul(pk[:], lhsT=khp, rhs=xpair_hh,
                                             start=st, stop=False, perf_mode=DR)
                            nc.tensor.matmul(pk[:], lhsT=khp, rhs=xpair_ll,
                                             start=False, stop=False, perf_mode=DR)
                            nc.tensor.matmul(pk[:], lhsT=klp, rhs=xpair_hh,
                                             start=False, stop=sp0, perf_mode=DR)
                    # V directly in natural [s, dh] layout: lhsT = x chunk
                    # pairs (stationary), rhs = wv chunk pairs; 3-term hi/lo
                    # fp8 DoubleRow like the q/k chains
                    for scl in range(4):
                        ssl = slice(scl * 128, (scl + 1) * 128)
                        for dcg in range(4):
                            xh, xl = xts[dcg]
                            xh2 = xh[:].rearrange("p (c m) -> p c m", c=4)
                            xl2 = xl[:].rearrange("p (c m) -> p c m", c=4)
                            for dcp in range(2):
                                dc = dcg * 4 + dcp * 2
                                st, sp0 = (dc == 0), (dc == ND - 2)
                                xph = xh2[:, dcp * 2:dcp * 2 + 2, ssl]
                                xpl = xl2[:, dcp * 2:dcp * 2 + 2, ssl]
                                vh = wv_hi[:, dc * 128:(dc + 2) * 128].rearrange(
                                    "p (c m) -> p c m", c=2)
                                vl = wv_lo[:, dc * 128:(dc + 2) * 128].rearrange(
                                    "p (c m) -> p c m", c=2)
                                pvs = pv[:, ssl]
                                nc.tensor.matmul(pvs, lhsT=xph, rhs=vh,
                                                 start=st, stop=False, perf_mode=DR)
                                nc.tensor.matmul(pvs, lhsT=xph, rhs=vl,
                                                 start=False, stop=False, perf_mode=DR)
                                nc.tensor.matmul(pvs, lhsT=xpl, rhs=vh,
                                                 start=False, stop=sp0, perf_mode=DR)
                    tcol = slice(tq5 * 512, (tq5 + 1) * 512)
                    emit_rope(tcol, pq, pk, pv)
            # ---------- phase B: attention + interleaved output projection --
            with tc.tile_pool(name="wop", bufs=1) as WOP, \
                 tc.tile_pool(name="ptp", bufs=4) as PTP, \
                 tc.tile_pool(name="rrp", bufs=2) as RRP, \
                 tc.tile_pool(name="osbp", bufs=3) as OSBP, \
                 tc.tile_pool(name="psSt", bufs=4, space="PSUM") as PSST, \
                 tc.tile_pool(name="psL", bufs=1, space="PSUM") as PSL, \
                 tc.tile_pool(name="psAv", bufs=1, space="PSUM") as PSAV, \
                 tc.tile_pool(name="psC", bufs=2, space="PSUM") as PSC:
                wo_hi = WOP.tile([128, HPC * D], f8, tag="woh")
                wo_lo = WOP.tile([128, HPC * D], f8, tag="wol")
                for sb, dr in ((wo_hi, woH), (wo_lo, woL)):
                    nc.sync.dma_start(
                        out=sb[:].rearrange("p (c n) -> p c n", c=HPC),
                        in_=dr[:, :].rearrange("(c p) n -> p c n", p=128))

                def emit_wo(tsb, tj):
                    tcx = (tsb * 512) // 128 + tj
                    avh = av_hi[:].rearrange("p (c m) -> p c m", c=HPC)
                    avl = av_lo[:].rearrange("p (c m) -> p c m", c=HPC)
                    wh = wo_hi[:].rearrange("p (c n) -> p c n", c=HPC)
                    wl = wo_lo[:].rearrange("p (c n) -> p c n", c=HPC)
                    tsl = slice(tcx * 128, (tcx + 1) * 128)
                    for dq in range(4):
                        dsl = slice(dq * 512, (dq + 1) * 512)
                        wo_ps = PSC.tile([128, D // 4], f32, tag="wops")
                        for hp in (0, 2):
                            hsl = slice(hp, hp + 2)
                            nc.tensor.matmul(
                                wo_ps[:], lhsT=avh[:, hsl, tsl],
                                rhs=wh[:, hsl, dsl],
                                start=(hp == 0), stop=False, perf_mode=DR)
                            nc.tensor.matmul(
                                wo_ps[:], lhsT=avh[:, hsl, tsl],
                                rhs=wl[:, hsl, dsl],
                                start=False, stop=False, perf_mode=DR)
                            nc.tensor.matmul(
                                wo_ps[:], lhsT=avl[:, hsl, tsl],
                                rhs=wh[:, hsl, dsl],
                                start=False, stop=(hp == 2), perf_mode=DR)
                        osb = OSBP.tile([128, D // 4], bf16, tag="osb")
                        if (tj + dq) % 2 == 0:
                            nc.vector.tensor_scalar_mul(osb[:], wo_ps[:], ODESC)
                        else:
                            nc.scalar.mul(osb[:], wo_ps[:], ODESC)
                        nc.sync.dma_start(
                            out=out[tsl, dsl], in_=osb[:])

                # deepest group first (best pipelining while nothing overlaps);
                # each group's output projection is emitted one group late so
                # it fills the next group's exp-latency stalls
                prev_tsb = None
                for tsb in [2, 3, 1, 0]:
                    n_sc = (tsb + 1) * 4
                    tg = slice(tsb * 512, (tsb + 1) * 512)
                    for h in range(HPC):
                        if prev_tsb is not None:
                            emit_wo(prev_tsb, h)
                        av_ps = PSAV.tile([128, 512], f32, tag="av")
                        l_ps = PSL.tile([128, 512], f32, tag="l")
                        for sc in range(n_sc):
                            sc_rel = sc - tsb * 4
                            c0 = max(sc_rel, 0) * 128   # first valid t col
                            # keep the moving operand >= 256 wide: float32r
                            # matmuls under 256 run at 1/4 rate
                            c0w = min(c0, 256)
                            nv = slice(c0w, 512)
                            tgn = slice(tsb * 512 + c0w, (tsb + 1) * 512)
                            st_ps = PSST.tile([128, 512], f32, tag="st")
                            nc.tensor.matmul(
                                st_ps[:, nv],
                                lhsT=KT[:, sc * 128:(sc + 1) * 128],
                                rhs=QT[h][:, tgn], start=True, stop=True)
                            if sc_rel >= 0:
                                blk = st_ps[:, c0:c0 + 128]
                                nc.vector.tensor_add(blk, blk, maskT[:])
                            if c0 > c0w:
                                # widened prefix is fully below the diagonal
                                nc.vector.tensor_scalar_add(
                                    st_ps[:, c0w:c0], st_ps[:, c0w:c0], NEG)
                            pt = PTP.tile([128, 512], fr, tag="pt")
                            nc.scalar.activation(
                                pt[:, nv], st_ps[:, nv],
                                mybir.ActivationFunctionType.Exp)
                            nc.tensor.matmul(
                                l_ps[0:1, nv], lhsT=ones[:], rhs=pt[:, nv],
                                start=(sc == 0), stop=(sc == n_sc - 1))
                            nc.tensor.matmul(
                                av_ps[:, nv],
                                lhsT=Vn[:, sc * 128:(sc + 1) * 128],
                                rhs=pt[:, nv], start=(sc == 0),
                                stop=(sc == n_sc - 1))
                        rr = RRP.tile([1, 512], fr, tag="rr")
                        nc.vector.reciprocal(rr[:], l_ps[0:1, :])
                        nc.vector.tensor_copy(AVT[h][:, tg], av_ps[:])
                        # rbc shares the l_ps bank (their lifetimes are
                        # naturally sequential), keeping wops free for wo
                        rbc = PSL.tile([128, 512], f32, tag="l", name="rbc")
                        nc.tensor.matmul(
                            rbc[:], lhsT=ones_r[:], rhs=rr[:],
                            start=True, stop=True)
                        nc.vector.tensor_mul(AVT[h][:, tg], AVT[h][:, tg], rbc[:])
                        # hi/lo fp8 decompose of av*AVSCALE for the fp8 wo
                        hsl8 = slice(h * T + tsb * 512, h * T + (tsb + 1) * 512)
                        nc.scalar.copy(av_hi[:, hsl8], AVT[h][:, tg])
                        nc.vector.tensor_sub(av_lo[:, hsl8], AVT[h][:, tg],
                                             av_hi[:, hsl8])
                    prev_tsb = tsb
                # last group's output projection runs at the tail
                for tj in range(4):
                    emit_wo(prev_tsb, tj)

    if split_waits:
        _split_multi_waits(nc, mybir)
    return nc


def _host_inputs(x, wq, wk, wv, wo):
    half = DH // 2
    inv = (1.0 / (ROPE_BASE ** (np.arange(half, dtype=np.float32) / half))).astype(np.float32)
    ang = np.arange(T, dtype=np.float32)[:, None] * inv[None, :]          # (T, 64)
    c = np.cos(ang).T.astype(np.float32)                                  # (64, T)
    s = np.sin(ang).T.astype(np.float32)
    cosT = np.ascontiguousarray(np.concatenate([c, c], axis=0))           # (128, T)
    sinT = np.ascontiguousarray(np.concatenate([s, s], axis=0))
    scale = np.float32(1.0 / np.sqrt(DH))
    import ml_dtypes
    bf = ml_dtypes.bfloat16
    f8 = ml_dtypes.float8_e4m3

    def hilo(a):
        hi = a.astype(f8)
        lo = (a - hi.astype(np.float32)).astype(f8)
        return np.ascontiguousarray(hi), np.ascontiguousarray(lo)

    in_maps = []
    for core in range(NCORES):
        b, kvh = core // KV, core % KV
        hlo = kvh * HPC * DH
        xb = x[b].T
        xh, xl = hilo(xb)
        wqh, wql = hilo((wq[hlo:hlo + HPC * DH, :] * (scale * WSCALE)).T)
        wkh, wkl = hilo(wk[kvh * DH:(kvh + 1) * DH, :].T * WSCALE)
        wvh, wvl = hilo(wv[kvh * DH:(kvh + 1) * DH, :].T * WSCALE)
        woh, wol = hilo(wo[:, hlo:hlo + HPC * DH].T * WSCALE)
        in_maps.append({
            "xH": xh, "xL": xl,
            "wqH": wqh, "wqL": wql,
            "wkH": wkh, "wkL": wkl,
            "wvH": wvh, "wvL": wvl,
            "woH": woh, "woL": wol,
            "cosT": cosT,
            "sinT": sinT,
        })
    return in_maps


def kernel(x, wq, wk, wv, wo):
    _ensure_path()
    from concourse.bass_utils import run_bass_kernel_spmd

    x = np.asarray(x, dtype=np.float32)
    wq = np.asarray(wq, dtype=np.float32)
    wk = np.asarray(wk, dtype=np.float32)
    wv = np.asarray(wv, dtype=np.float32)
    wo = np.asarray(wo, dtype=np.float32)

    if "nc" not in _cache:
        _cache["nc"] = _build()
    nc = _cache["nc"]

    in_maps = _host_inputs(x, wq, wk, wv, wo)
    res = run_bass_kernel_spmd(nc, in_maps, list(range(NCORES)))
    out = np.zeros((B, T, D), dtype=np.float32)
    for core in range(NCORES):
        b = core // KV
        out[b] += res.results[core]["out"].astype(np.float32)
    return out
